# revision 19
# baseline (speedup 1.0000x reference)
"""Conv2d (32,128,56,56) x (256,128,3,3) pad=1 -> (32,256,56,56) on 8 trn2 cores.

Strategy: data-parallel over batch (4 images/core). On each core the conv is
9 accumulating matmuls per output tile: contraction over C=128 (partition
dim), stationary operand = per-tap weight slab [C=128, O_half=128], moving
operand = shifted window of the zero-padded input rows [C=128, 8 rows x 56].
PSUM accumulates the 9 taps; DVE adds bias while evacuating to SBUF; DMA out.

Matmul operands are bf16: fp32 LDWEIGHTS (187 ns, no FWL) doesn't hide
behind the 187 ns moving stream (measured 210 ns/MM), while bf16 gets FWL
and sustains the 189 ns streaming floor. bf16 rounding error across the
K=1152 contraction is ~2e-3 rel, well inside the 2e-2 gate. Output is
stored bf16 (halves store traffic + DVE write time); host converts back.

Startup: the framework preamble ends ~7.2 us; DMA issue order is hand-picked
so the first output tile's operands (x rows 0-9 of image 0 + taps 0-2 of the
first weight half) land ~9.5 us, with a short burst of N=128 warmup matmuls
filling the PE pipe (and the HAM activity window) until then.
"""

import os
import sys

for _p in ("/opt/trn_rl_repo", "/root/.axon_site/_ro/trn_rl_repo"):
    if os.path.isdir(_p) and _p not in sys.path:
        sys.path.insert(0, _p)

import numpy as np
import ml_dtypes

BF16 = ml_dtypes.bfloat16

N_CORES = 8
B, C, H, W = 32, 128, 56, 56
O, KH, KW = 256, 3, 3
BPC = B // N_CORES          # images per core
HP, WP = H + 2, W + 2       # padded spatial
WPAD = 64                   # SBUF/DRAM row pitch: 128 B rows keep every
                            # tap window's row phase uniform (16B-aligned
                            # for kw=0) for the PE moving-operand fetch
ROWS = 8                    # output rows per matmul chunk
NCH = H // ROWS             # chunks per image
NF = ROWS * W               # matmul free dim (448 <= 512 fp32 PSUM bank)
N_WARM = 26                 # N=128 warmup matmuls to fill preamble->data gap

_cached_nc = None


def _build_program():
    import concourse.tile as tile
    from concourse import bacc, mybir

    nc = bacc.Bacc(
        "TRN2", target_bir_lowering=False, debug=False, num_devices=N_CORES
    )
    f32 = mybir.dt.float32
    bf16 = mybir.dt.bfloat16

    xp = nc.dram_tensor("xp", (C, BPC, HP, WPAD), bf16, kind="ExternalInput").ap()
    wt = nc.dram_tensor("wt", (C, O // C, KH * KW, 128), bf16, kind="ExternalInput").ap()
    bias = nc.dram_tensor("bias", (C, O // C), f32, kind="ExternalInput").ap()
    out = nc.dram_tensor("out", (BPC * O, H * W), bf16, kind="ExternalOutput").ap()

    with tile.TileContext(nc) as tc:
        with (
            tc.tile_pool(name="consts", bufs=1) as consts,
            tc.tile_pool(name="xpool", bufs=1) as xpool,
            tc.tile_pool(name="opool", bufs=16) as opool,
            tc.tile_pool(name="psum", bufs=5, space="PSUM") as pspool,
        ):
            # PE prewarm: short dummy matmuls while the first DMAs stream in,
            # so the PE pipe is busy from the end of the preamble and the HAM
            # clock gate sees sustained activity. N=128 keeps each one cheap
            # so overshoot past data-arrival costs little.
            warm_x = consts.tile([C, 128], bf16, tag="warm_x")
            nc.gpsimd.memset(warm_x[:], 0.0)
            warm_ps = pspool.tile([128, 128], f32, tag="warm_ps", bufs=1)
            for _ in range(N_WARM):
                nc.tensor.matmul(warm_ps[:], warm_x[:], warm_x[:], start=True, stop=True)

            # Loads are split across the TWO HWDGE rings so their ~0.65 us
            # per-DMA issue costs run in parallel: x on the sync ring, weights
            # + bias on the scalar (Activation) ring. Each ring issues in
            # hand-picked FIFO order so the first output tile's operands
            # (image-0 rows 0-9 + taps 0-2 of output-half 0) land first.
            # Image 0 is split in row bands (chunk ch reads padded rows
            # 8ch..8ch+9, so band boundaries at 10,18,26,... make chunk ch
            # ready after band ch lands). Stores go on the scalar ring, after
            # all its loads, so a not-yet-ready store never head-of-line
            # blocks an input load.
            w_sb = consts.tile([C, O // C, KH * KW, 128], bf16)
            bias_sb = consts.tile([C, O // C], f32)
            x_sbs = []
            for i in range(BPC):
                x_sb = xpool.tile([C, HP, WPAD], bf16, tag=f"x{i}")
                x_sbs.append(x_sb)
            bands = [(0, 10), (10, 18), (18, 26), (26, 34), (34, 42), (42, 50), (50, HP)]
            for r0, r1 in bands:
                nc.sync.dma_start(x_sbs[0][:, r0:r1], xp[:, 0, r0:r1])
            for i in range(1, BPC):
                nc.sync.dma_start(x_sbs[i][:], xp[:, i])
            nc.scalar.dma_start(w_sb[:, 0, 0:3], wt[:, 0, 0:3])
            nc.scalar.dma_start(w_sb[:, 0, 3:6], wt[:, 0, 3:6])
            nc.scalar.dma_start(w_sb[:, 0, 6:9], wt[:, 0, 6:9])
            nc.scalar.dma_start(bias_sb[:], bias[:])
            nc.scalar.dma_start(w_sb[:, 1], wt[:, 1])

            # Opening group (image 0, output-half 0, chunks 0+1): interleave
            # the two chunks in tap-triples so the first 6 matmuls need only
            # taps 0-2 and bands 0-1 — each later weight-triple's deadline
            # then falls after its measured DMA landing, keeping the PE
            # stall-free through the HAM activity window.
            ps01 = [
                pspool.tile([128, NF], f32, name=f"ps01_{c}", bufs=1)
                for c in range(2)
            ]
            for tg in range(KH):
                for ch in range(2):
                    for t in (3 * tg, 3 * tg + 1, 3 * tg + 2):
                        kh, kw = divmod(t, KW)
                        y0 = ch * ROWS
                        rhs = x_sbs[0][:, y0 + kh : y0 + kh + ROWS, kw : kw + W]
                        nc.tensor.matmul(
                            ps01[ch][:], w_sb[:, 0, t, :], rhs,
                            start=(t == 0), stop=(t == KH * KW - 1),
                        )
            for ch in range(2):
                o_sb = opool.tile([128, NF], bf16)
                nc.vector.tensor_scalar_add(o_sb[:], ps01[ch][:], bias_sb[:, 0:1])
                nc.scalar.dma_start(out[0:128, ch * NF : (ch + 1) * NF], o_sb[:])

            for i in range(BPC):
                for oh in range(O // C):
                    for ch in range(NCH):
                        if i == 0 and oh == 0 and ch < 2:
                            continue
                        y0 = ch * ROWS
                        last = (i == BPC - 1) and (oh == O // C - 1) and (ch == NCH - 1)
                        # The final chunk is computed as two 4-row halves so
                        # the critical tail (last matmul -> evac -> store) is
                        # half as long; everywhere else one 8-row chunk.
                        subs = [(0, ROWS)] if not last else [(0, 4), (4, ROWS)]
                        for r_lo, r_hi in subs:
                            nr = r_hi - r_lo
                            nf = nr * W
                            ps = pspool.tile([128, nf], f32)
                            for t in range(KH * KW):
                                kh, kw = divmod(t, KW)
                                rhs = x_sbs[i][
                                    :, y0 + r_lo + kh : y0 + r_lo + kh + nr, kw : kw + W
                                ]
                                lhsT = w_sb[:, oh, t, :]
                                nc.tensor.matmul(
                                    ps[:], lhsT, rhs,
                                    start=(t == 0), stop=(t == KH * KW - 1),
                                )
                            o_sb = opool.tile([128, nf], bf16)
                            nc.vector.tensor_scalar_add(
                                o_sb[:], ps[:], bias_sb[:, oh : oh + 1]
                            )
                            r0 = i * O + oh * 128
                            c0 = ch * NF + r_lo * W
                            nc.scalar.dma_start(
                                out[r0 : r0 + 128, c0 : c0 + nf], o_sb[:]
                            )
    nc.compile()
    return nc


def _get_program():
    global _cached_nc
    if _cached_nc is None:
        _cached_nc = _build_program()
    return _cached_nc


def _prep_inputs(x, kernels, biases):
    """Host-side shard + layout prep. Returns list of per-core input maps."""
    x = np.ascontiguousarray(x, dtype=np.float32)
    kernels = np.ascontiguousarray(kernels, dtype=np.float32)
    biases = np.ascontiguousarray(biases, dtype=np.float32)

    xpad = np.zeros((B, C, HP, WPAD), dtype=BF16)
    xpad[:, :, 1 : H + 1, 1 : W + 1] = x.astype(BF16)

    # wt[c, oh, t, o'] = kernels[oh*128 + o', c, kh, kw]
    wt = np.ascontiguousarray(
        kernels.astype(BF16)
        .transpose(1, 2, 3, 0)
        .reshape(C, KH * KW, O // C, 128)
        .transpose(0, 2, 1, 3)
    )
    # bias_sb[o', h] = biases[h*128 + o']
    bias2 = np.ascontiguousarray(biases.reshape(O // C, C).T)

    in_maps = []
    for core in range(N_CORES):
        xc = np.ascontiguousarray(
            xpad[core * BPC : (core + 1) * BPC].transpose(1, 0, 2, 3)
        )
        in_maps.append({"xp": xc, "wt": wt, "bias": bias2})
    return in_maps


def _run(in_maps, trace=False, **kw):
    from concourse.bass_utils import run_bass_kernel_spmd

    nc = _get_program()
    return run_bass_kernel_spmd(
        nc, in_maps, core_ids=list(range(N_CORES)), trace=trace, **kw
    )


def kernel(x, kernels, biases):
    res = _run(_prep_inputs(x, kernels, biases))
    outs = [
        r["out"].astype(np.float32).reshape(BPC, O, H, W) for r in res.results
    ]
    return np.concatenate(outs, axis=0)
